# revision 3
# baseline (speedup 1.0000x reference)
"""CLIP-style contrastive loss on 8 Trainium2 NeuronCores.

Math: with labels = eye(B), the Keras CCE(prob, eye) loss only depends on the
diagonal of the softmax probabilities:
    sum_ij prob_ij * logclip_ij = tr * log(1-eps) + (B - tr) * log(eps)
where tr = trace(prob). And since |logits| <= exp(temperature) (cosine
similarities), softmax needs no max subtraction; prob_ii = E_ii / rowsum(E)
with E = exp(scale * S), S = l2norm(left) @ l2norm(right).T.

Sharding: 4x2 grid over the BxB similarity matrix. Core (p, q) owns
left rows [1024p, 1024p+1024) x right rows [2048q, 2048q+2048), computes its
S block via bf16 matmul (d-major operands built with xbar DMA transpose via a
DRAM scratch round-trip), then exp, per-block row sums (fused in ScalarE
accum), per-block column sums (ones-vector matmul) and the diagonal candidates
(identity-mask tensor_tensor_reduce). Host combines the tiny partial sums.
"""

import math
import os
import numpy as np

import concourse.bass as bass
import concourse.mybir as mybir
import concourse.tile as tile
from concourse import bacc
from concourse.bass import ds, ts
from concourse.masks import make_identity

B = 4096
D = 1024
EPS = 1e-7
WEIGHT = 1.0

PGRID = 4  # row groups (left)
QGRID = 2  # col groups (right)
LROWS = B // PGRID   # 1024 left rows per core
RROWS = B // QGRID   # 2048 right rows per core
KT = D // 128        # 8 contraction k-tiles
MT = LROWS // 128    # 8 m row-tiles
NCHUNK = 2           # right processed in chunks of 1024 rows
CHR = RROWS // NCHUNK

AF = mybir.ActivationFunctionType
F32 = mybir.dt.float32
BF16 = mybir.dt.bfloat16


def _build_body(tc, lblk, rblk, temp, rowsum_o, colsum_o, diag_o):
    nc = tc.nc
    from contextlib import ExitStack

    with ExitStack() as ctx:
        const_pool = ctx.enter_context(tc.tile_pool(name="const", bufs=1))
        small = ctx.enter_context(tc.tile_pool(name="small", bufs=1))
        nat_pool = ctx.enter_context(tc.tile_pool(name="nat", bufs=10))
        sq_pool = ctx.enter_context(tc.tile_pool(name="sq", bufs=2))
        nrm_pool = ctx.enter_context(tc.tile_pool(name="nrm", bufs=4))
        lnT_pool = ctx.enter_context(tc.tile_pool(name="lnT", bufs=KT))
        rnT_pool = ctx.enter_context(tc.tile_pool(name="rnT", bufs=KT))
        e_pool = ctx.enter_context(tc.tile_pool(name="E", bufs=MT))
        dram_pool = ctx.enter_context(tc.tile_pool(name="scr", bufs=2, space="DRAM"))
        ps_mm = ctx.enter_context(tc.tile_pool(name="psmm", bufs=4, space="PSUM"))
        ps_cs = ctx.enter_context(tc.tile_pool(name="pscs", bufs=2, space="PSUM"))
        ps_es = ctx.enter_context(tc.tile_pool(name="pses", bufs=1, space="PSUM"))

        # ---- constants ----
        eye = const_pool.tile([128, 128], BF16, tag="eye")
        make_identity(nc, eye[:])
        ones_row = const_pool.tile([1, 128], F32, tag="ones_row")  # escale bcast lhsT
        nc.vector.memset(ones_row[:], 1.0)
        ones_col = const_pool.tile([128, 1], BF16, tag="ones_col")  # colsum lhsT
        nc.vector.memset(ones_col[:], 1.0)

        # ---- escale = exp(temperature), broadcast to all 128 partitions ----
        t_sb = small.tile([1, 1], F32, tag="t_sb")
        nc.sync.dma_start(t_sb[:], temp.rearrange("(a b) -> a b", a=1))
        esc11 = small.tile([1, 1], F32, tag="esc11")
        nc.scalar.activation(esc11[:], t_sb[:], AF.Exp)
        esc_ps = ps_es.tile([128, 1], F32, tag="escps")
        nc.tensor.matmul(esc_ps[:], ones_row[:], esc11[:], start=True, stop=True)
        escale = small.tile([128, 1], F32, tag="escale")
        nc.vector.tensor_copy(escale[:], esc_ps[:])

        # ---- accumulators ----
        rowacc = small.tile([128, MT * 2 * NCHUNK], F32, tag="rowacc")
        diagacc = small.tile([128, MT * 2], F32, tag="diagacc")
        colsb = small.tile([1, RROWS], F32, tag="colsb")

        # ---- left tower: load, sqsum, normalize+cast, DRAM round-trip transpose ----
        sqL = small.tile([128, MT], F32, tag="sqL")
        ltiles = []
        for mt in range(MT):
            lt = nat_pool.tile([128, D], F32, tag="nat")
            nc.sync.dma_start(lt[:], lblk[ts(mt, 128), :])
            sqd = sq_pool.tile([128, D], F32, tag="sq")
            nc.scalar.activation(sqd[:], lt[:], AF.Square,
                                 accum_out=sqL[:, ds(mt, 1)])
            ltiles.append(lt)
        mxL = small.tile([128, MT], F32, tag="mxL")
        nc.vector.tensor_scalar_max(mxL[:], sqL[:], EPS)
        srL = small.tile([128, MT], F32, tag="srL")
        nc.scalar.sqrt(srL[:], mxL[:])
        invL = small.tile([128, MT], F32, tag="invL")
        nc.vector.reciprocal(invL[:], srL[:])

        lscr = dram_pool.tile([LROWS, D], BF16, tag="lscr")
        for mt in range(MT):
            nb = nrm_pool.tile([128, D], BF16, tag="nrm")
            nc.vector.tensor_scalar_mul(nb[:], ltiles[mt][:], invL[:, ds(mt, 1)])
            nc.sync.dma_start(lscr[ts(mt, 128), :], nb[:])
        lnTs = []
        for k in range(KT):
            lt = lnT_pool.tile([128, LROWS], BF16, tag="lnT")
            nc.sync.dma_start_transpose(lt[:], lscr[:, ts(k, 128)])
            lnTs.append(lt)

        # ---- right tower + matmul + exp, chunked over right rows ----
        rnTs = [rnT_pool.tile([128, RROWS], BF16, tag="rnT", name=f"rnT{k}") for k in range(KT)]
        etiles = [e_pool.tile([128, RROWS], BF16, tag="E", name=f"E{m}") for m in range(MT)]
        sqR = small.tile([128, NCHUNK * CHR // 128], F32, tag="sqR")

        for rc in range(NCHUNK):
            rts = CHR // 128  # row tiles in this chunk
            rtiles = []
            for rt in range(rts):
                rtile = nat_pool.tile([128, D], F32, tag="nat")
                nc.sync.dma_start(rtile[:], rblk[ds(rc * CHR + rt * 128, 128), :])
                sqd = sq_pool.tile([128, D], F32, tag="sq")
                nc.scalar.activation(sqd[:], rtile[:], AF.Square,
                                     accum_out=sqR[:, ds(rc * rts + rt, 1)])
                rtiles.append(rtile)
            mxR = small.tile([128, rts], F32, tag=f"mxR{rc}")
            nc.vector.tensor_scalar_max(mxR[:], sqR[:, ds(rc * rts, rts)], EPS)
            srR = small.tile([128, rts], F32, tag=f"srR{rc}")
            nc.scalar.sqrt(srR[:], mxR[:])
            invR = small.tile([128, rts], F32, tag=f"invR{rc}")
            nc.vector.reciprocal(invR[:], srR[:])

            rscr = dram_pool.tile([CHR, D], BF16, tag="rscr")
            for rt in range(rts):
                nb = nrm_pool.tile([128, D], BF16, tag="nrm")
                nc.vector.tensor_scalar_mul(nb[:], rtiles[rt][:], invR[:, ds(rt, 1)])
                nc.sync.dma_start(rscr[ts(rt, 128), :], nb[:])
            for k in range(KT):
                nc.sync.dma_start_transpose(rnTs[k][:, ds(rc * CHR, CHR)],
                                            rscr[:, ts(k, 128)])

            # matmul S block (chunk columns) + fused exp/rowsum
            for mt in range(MT):
                pss = [ps_mm.tile([128, 512], F32, tag="ps", name=f"ps{mt}_{j}") for j in range(2)]
                for k in range(KT):
                    for j in range(2):
                        nc.tensor.matmul(
                            pss[j][:],
                            lnTs[k][:, ts(mt, 128)],
                            rnTs[k][:, ds(rc * CHR + j * 512, 512)],
                            start=(k == 0), stop=(k == KT - 1),
                        )
                for j in range(2):
                    nc.scalar.activation(
                        etiles[mt][:, ds(rc * CHR + j * 512, 512)],
                        pss[j][:], AF.Exp, scale=escale[:, 0:1],
                        accum_out=rowacc[:, ds(mt * 2 * NCHUNK + rc * 2 + j, 1)],
                    )

            # column sums of exp over this chunk (partition reduce via ones-matmul)
            for j in range(2):
                cps = ps_cs.tile([1, 512], F32, tag="cs")
                for mt in range(MT):
                    nc.tensor.matmul(
                        cps[:], ones_col[:],
                        etiles[mt][:, ds(rc * CHR + j * 512, 512)],
                        start=(mt == 0), stop=(mt == MT - 1),
                    )
                nc.vector.tensor_copy(colsb[:, ds(rc * CHR + j * 512, 512)], cps[:])

            # diagonal candidates at column offset 1024*rc
            for mt in range(MT):
                dscr = sq_pool.tile([128, 128], F32, tag="dscr",
                                    name=f"dscr{rc}_{mt}")
                nc.vector.tensor_mul(
                    dscr[:], etiles[mt][:, ds(rc * CHR + mt * 128, 128)], eye[:])
                nc.vector.tensor_reduce(
                    diagacc[:, ds(mt * 2 + rc, 1)], dscr[:],
                    axis=mybir.AxisListType.X, op=mybir.AluOpType.add)

        # ---- finalize outputs ----
        rs = small.tile([128, MT], F32, tag="rs")
        nc.vector.tensor_reduce(
            rs[:], rowacc[:].rearrange("p (m c) -> p m c", c=2 * NCHUNK),
            axis=mybir.AxisListType.X, op=mybir.AluOpType.add,
        )
        nc.sync.dma_start(rowsum_o[:], rs[:])
        nc.sync.dma_start(colsum_o.rearrange("(a c) -> a c", a=1), colsb[:])
        nc.sync.dma_start(diag_o[:], diagacc[:])


_CACHED = {}


def _get_program():
    if "nc" in _CACHED:
        return _CACHED["nc"]
    nc = bacc.Bacc("TRN2", target_bir_lowering=False, debug=False,
                   num_devices=PGRID * QGRID)
    lblk = nc.dram_tensor("lblk", [LROWS, D], F32, kind="ExternalInput").ap()
    rblk = nc.dram_tensor("rblk", [RROWS, D], F32, kind="ExternalInput").ap()
    temp = nc.dram_tensor("temp", [1], F32, kind="ExternalInput").ap()
    rowsum_o = nc.dram_tensor("rowsum", [128, MT], F32, kind="ExternalOutput").ap()
    colsum_o = nc.dram_tensor("colsum", [RROWS], F32, kind="ExternalOutput").ap()
    diag_o = nc.dram_tensor("diag", [128, MT * 2], F32, kind="ExternalOutput").ap()
    with tile.TileContext(nc) as tc:
        _build_body(tc, lblk, rblk, temp, rowsum_o, colsum_o, diag_o)
    nc.compile()
    _CACHED["nc"] = nc
    return nc


def _run(inputs, trace=False):
    from concourse.bass_utils import run_bass_kernel_spmd

    nc = _get_program()
    left = np.ascontiguousarray(inputs["left"], dtype=np.float32)
    right = np.ascontiguousarray(inputs["right"], dtype=np.float32)
    temp = np.ascontiguousarray(inputs["temperature"], dtype=np.float32)

    in_maps = []
    for p in range(PGRID):
        for q in range(QGRID):
            in_maps.append({
                "lblk": left[p * LROWS:(p + 1) * LROWS],
                "rblk": right[q * RROWS:(q + 1) * RROWS],
                "temp": temp,
            })
    res = run_bass_kernel_spmd(nc, in_maps, core_ids=list(range(PGRID * QGRID)),
                               trace=trace)
    return res


def _combine(results):
    # rowsum: partial over q; global row = 1024p + 128mt + part
    rowsum = np.zeros(B, dtype=np.float64)
    colsum = np.zeros(B, dtype=np.float64)
    diag = np.zeros(B, dtype=np.float64)
    for p in range(PGRID):
        for q in range(QGRID):
            r = results[p * QGRID + q]
            rs = r["rowsum"].astype(np.float64)  # [128, MT]
            rowsum[p * LROWS:(p + 1) * LROWS] += rs.T.reshape(-1)
            colsum[q * RROWS:(q + 1) * RROWS] += r["colsum"].astype(np.float64)
            delta = LROWS * p - RROWS * q
            if delta in (0, CHR):
                a = delta // CHR
                d = r["diag"].astype(np.float64).reshape(128, MT, 2)[:, :, a]
                diag[p * LROWS:(p + 1) * LROWS] = d.T.reshape(-1)
    tr_l = float(np.sum(diag / rowsum))
    tr_r = float(np.sum(diag / colsum))
    log_eps = math.log(EPS)
    log_1meps = math.log(1.0 - EPS)
    loss_l = -(tr_l * log_1meps + (B - tr_l) * log_eps)
    loss_r = -(tr_r * log_1meps + (B - tr_r) * log_eps)
    loss = WEIGHT * (loss_l + loss_r) / 2.0 / B
    return np.asarray(loss, dtype=np.float32)


def kernel(**inputs):
    res = _run(inputs, trace=False)
    return _combine(res.results)


def kernel_traced(**inputs):
    res = _run(inputs, trace=True)
    return _combine(res.results), res
